# revision 8
# baseline (speedup 1.0000x reference)
"""Trainium2 Bass kernel for nn_NXROAttentiveModel (v2).

Exact algebraic reduction of the reference (rank-1 QKV => 3-term masked
softmax; Fourier emb folded into 5 basis matrices host-side). Pure data
parallel over 8 cores; per core bc=32768 = 128 partitions x 256 samples,
sample s = p*256 + col.

Per-rep structure (engines balanced ~16-17us each in CoreSim;
~16.9us/rep steady-state, 91% SP occupancy):
  - dxdt via PE, two block-halves: zx [128, 5*11*128] f16 slot-major
    blocks (12 samples per 128-col block, col = v*12+slot) so the
    4 feature muls run in DVE 2x_1p mode; k0 slot-shuffle is an ACT
    copy. One XBAR transpose per half -> zxT; 5 stationaries W5
    [128,120] accumulate into PSUM [120,1408]; one ACT evac -> dxT f16;
    XBAR back -> dxsm (block col = slot*10+u). Evac/back-transpose and
    the final add + store are software-pipelined one rep behind.
  - attention: 2 chunks of 128 samples; exp args f16 (softmax ratio is
    robust to arg rounding), exp outputs bf16 (range), numerator chain
    f32 on Pool (broadcast APs cannot use DVE 2x anyway), denominator
    bf16 on DVE 2x; 1/den via exp(-ln(den)) keeps Exp/Ln/Square/Copy in
    one ACT table (natural_log_exp) => no table reloads in steady state.
  - alpha folded into xal = x*sigmoid(emb@aw)*c_vo once per execution;
    x01e = x_{0,1} broadcast-expanded host-side (input xe) so a01 runs
    2x on DVE; input DMAs split across the SP and ACT HWDGE queues.
  - fp16 in/out tensors halve HBM + host-link traffic; the final add
    (Pool) writes f16 t3 in-place and a plain SP DMA stores it — the
    store must live on SP: on ACT/Pool it head-of-line blocks the next
    rep's shuffles/attention ops behind the final-add dependency. PSUM
    evac is split ACT/DVE; evac, back-transpose, final add and store
    for rep r are emitted one rep behind.
Max |err| vs f64 reference ~2.7e-3 of absmax on HW.
"""
import sys

sys.path.insert(0, "/opt/trn_rl_repo")

import math
import numpy as np
import concourse.bass as bass
import concourse.mybir as mybir
from concourse import tile

V = 10
P = 128
AF = mybir.ActivationFunctionType
OP = mybir.AluOpType
F32 = mybir.dt.float32
BF16 = mybir.dt.bfloat16
F16 = mybir.dt.float16
PI = math.pi

B_FULL = 262144
NCORES = 8
BC = B_FULL // NCORES          # 32768 per core
GALL = BC // P                 # 256 samples per partition
SLOT = 12                      # samples per 128-col block
NBF = GALL // SLOT             # 21 full blocks
TAIL = GALL - NBF * SLOT       # 4 tail samples (block 21, slots 0..3)
NB = NBF + 1                   # 22 blocks
CPK = NB * P                   # cols per k-section = 2816
NSEC = 5
NCOL = NSEC * CPK              # zx cols = 14080
GA = 128                       # samples per attention chunk
NCH = GALL // GA               # 2 chunks


def _hoist_excess_waits(nc, max_waits=1):
    """Walrus codegen allows only one fused sem-wait per compute instruction."""
    n = 0
    for blk in nc.main_func.blocks:
        il = blk.instructions
        i = 0
        while i < len(il):
            ins = il[i]
            si = ins.sync_info
            if (si is not None and si.on_wait and len(si.on_wait) > max_waits
                    and not isinstance(ins, mybir.InstEventSemaphore)):
                keep = list(si.on_wait[-max_waits:])
                hoist = list(si.on_wait[:-max_waits])
                for w in hoist:
                    nop = mybir.InstEventSemaphore(name=f"hoistw-{n}", ins=[], outs=[])
                    n += 1
                    nop.engine = ins.engine
                    nop.sync_info = mybir.SyncInfo(on_wait=[w], on_update=[])
                    nc.register_instruction(nop, overwrite=True)
                    il.insert(i, nop)
                    i += 1
                ins.sync_info = mybir.SyncInfo(
                    on_wait=keep, on_update=list(si.on_update))
            i += 1
    return n


def build_program(bc, c_qk, c_vo, aw1, aw2, aw3, aw4, reps=1, pool_cfg=None):
    if pool_cfg is None:
        pool_cfg = {"s01", "ss", "pd", "nsp", "q2"}
    assert bc == BC
    nc = bass.Bass()
    xb_d = nc.dram_tensor("xb", [bc, V], F16, kind="ExternalInput")
    xe_d = nc.dram_tensor("xe", [bc, 2 * V], F16, kind="ExternalInput")
    t_d = nc.dram_tensor("t", [bc], F32, kind="ExternalInput")
    w5_d = nc.dram_tensor("w5", [P, NSEC * 120], F16, kind="ExternalInput")
    cst_d = nc.dram_tensor("cst", [4], F32, kind="ExternalInput")
    out_d = nc.dram_tensor("out", [bc, V], F16, kind="ExternalOutput")

    def dram_ap(d, off, dims):
        return bass.AP(d.tensor if hasattr(d, "tensor") else d, off, dims)

    def tap(tl, off, dims):
        return bass.AP(tl.tensor, off, [tl.ap[0]] + dims)

    with tile.TileContext(nc) as tc:
        with (
            tc.tile_pool(name="res", bufs=1) as rp,
            tc.tile_pool(name="att", bufs=2) as ap_,
            tc.tile_pool(name="att1", bufs=1) as ap1,
            tc.tile_pool(name="io", bufs=2) as iop,
            tc.tile_pool(name="ps", bufs=1, space="PSUM") as psp,
        ):
            # ---- resident tiles ----
            xb = rp.tile([P, GALL * V], F16)      # sample-major x (v-major)
            xal = rp.tile([P, GALL * V], F16)     # x * alpha * c_vo
            x01e = rp.tile([P, GALL * 2 * V], F16)  # x_u bcast over v
            dxT = rp.tile([P, CPK], F16)          # matmul out, feat-major
            NBH = 11                      # blocks per half
            CPKH = NBH * P                # 1408 cols per k-section per half
            NCOLH = NSEC * CPKH           # 7040
            zxhs = [rp.tile([P, NCOLH], F16, name=f"zxh{h}") for h in range(2)]
            zxThs = [rp.tile([P, NCOLH], F16, name=f"zxTh{h}") for h in range(2)]
            emb = rp.tile([P, 4 * GALL], F16)     # c_k planes, bf16
            al2b = rp.tile([P, GALL], F16)
            em4 = rp.tile([P, 4 * GALL], F32)      # f32 planes (trig out)
            sh = rp.tile([P, GALL], F32)
            tmp1 = rp.tile([P, GALL], F32)
            tmp2 = rp.tile([P, GALL], F32)
            w5 = rp.tile([P, NSEC * 120], F16)
            cst = rp.tile([P, 4], F32)

            # ---- one-time: input DMAs + pad memsets ----
            nc.sync.dma_start(
                out=xb[:], in_=dram_ap(xb_d, 0, [[GALL * V, P], [1, GALL * V]]))
            nc.sync.dma_start(out=w5[:], in_=dram_ap(
                w5_d, 0, [[NSEC * 120, P], [1, NSEC * 120]]))
            nc.scalar.dma_start(
                out=x01e[:],
                in_=dram_ap(xe_d, 0, [[GALL * 2 * V, P], [1, GALL * 2 * V]]))
            nc.scalar.dma_start(
                out=tmp1[:], in_=dram_ap(t_d, 0, [[GALL, P], [1, GALL]]))
            nc.scalar.dma_start(out=cst[:], in_=dram_ap(cst_d, 0, [[0, P], [1, 4]]))
            # dxT rows 120..127 are never written by evac; zero once so the
            # back-transpose reads defined data
            nc.gpsimd.memset(dxT[96:128, :], 0.0)
            # zx pad cols (120..127 of each block, all sections) + tail slots
            for h in range(2):
                nc.gpsimd.memset(
                    tap(zxhs[h], 120, [[CPKH, NSEC], [P, NBH], [1, 8]]), 0.0)
            nc.gpsimd.memset(
                tap(zxhs[1], (NBH - 1) * P + TAIL,
                    [[CPKH, NSEC], [SLOT, V], [1, SLOT - TAIL]]), 0.0)

            pend_ps = None   # psth pair from previous rep, evac deferred
            pend_t3 = None   # t3 from previous rep
            for rep in range(reps):
                # ---- deferred evac + back-transposes for rep-1 ----
                prev = None
                if pend_ps is not None:
                    prev = (pend_t3, emit_evac_backs(pend_ps))
                    pend_ps = pend_t3 = None

                if rep == 0:
                    # rep-0 shuffles first so the trig chain cannot
                    # head-of-line block them on ACT
                    emit_shuffle(0)
                    emit_shuffle(1)
                def emit_shuffle(h):
                nfull = NBH if h == 0 else NBF - NBH
                nc.scalar.copy(
                    tap(zxhs[h], 0, [[P, nfull], [SLOT, V], [1, SLOT]]),
                    tap(xb, h * NBH * SLOT * V,
                        [[SLOT * V, nfull], [1, V], [V, SLOT]]),
                )
                if h == 1 and TAIL:
                    nc.scalar.copy(
                        tap(zxhs[h], nfull * P, [[SLOT, V], [1, TAIL]]),
                        tap(xb, NBF * SLOT * V, [[1, V], [V, TAIL]]),
                    )

            def emit_ablk():
                # exp-arg builds, both chunks back-to-back (DVE 2x)
                ablks = []
                for ch in range(NCH):
                    ao = ch * GA * V
                    ablk = ap_.tile([P, GA * 28], F16, tag="ablk")
                    # a01[g,u,v] = x_v * x_u  (2x via pre-expanded x01e)
                    nc.vector.tensor_mul(
                        out=tap(ablk, 0, [[28, GA], [V, 2], [1, V]]),
                        in0=tap(xb, ao, [[V, GA], [0, 2], [1, V]]),
                        in1=tap(x01e, 2 * ao, [[2 * V, GA], [V, 2], [1, V]]),
                    )
                    # xsq[g, v>=2] = x_v^2   (2x)
                    nc.vector.tensor_mul(
                        out=tap(ablk, 2 * V, [[28, GA], [1, 8]]),
                        in0=tap(xb, ao + 2, [[V, GA], [1, 8]]),
                        in1=tap(xb, ao + 2, [[V, GA], [1, 8]]),
                    )
                    ablks.append(ablk)
                return ablks

            def emit_eblk(ablks):
                eblks = []
                for ch in range(NCH):
                    eblk = ap_.tile([P, GA * 28], BF16, tag="eblk")
                    nc.scalar.activation(eblk[:], ablks[ch][:], AF.Exp,
                                         scale=c_qk)
                    eblks.append(eblk)
                return eblks

        # ---- one-time: trig + emb planes + alpha + xal ----
            # tmp1 holds t; sh = sin(pi t); tmp2 = cos(pi t)
            nc.scalar.activation(sh[:], tmp1[:], AF.Sin, scale=PI)
            nc.scalar.activation(tmp2[:], tmp1[:], AF.Sin, scale=-PI,
                                 bias=cst[:, 1:2])
            eA = tap(em4, 0 * GALL, [[1, GALL]])
            eB = tap(em4, 1 * GALL, [[1, GALL]])
            eA2 = tap(em4, 2 * GALL, [[1, GALL]])
            eAB = tap(em4, 3 * GALL, [[1, GALL]])
            nc.scalar.activation(eA, sh[:], AF.Square)
            nc.vector.tensor_mul(out=eB, in0=sh[:], in1=tmp2[:])
            nc.scalar.activation(eA2, eA, AF.Square)
            nc.vector.tensor_mul(out=eAB, in0=eA, in1=eB)
            nc.vector.scalar_tensor_tensor(out=tmp1[:], in0=eA, scalar=aw1,
                                           in1=eA, op0=OP.mult, op1=OP.bypass)
            nc.vector.scalar_tensor_tensor(out=tmp2[:], in0=eB, scalar=aw2,
                                           in1=tmp1[:], op0=OP.mult, op1=OP.add)
            nc.vector.scalar_tensor_tensor(out=tmp1[:], in0=eA2, scalar=aw3,
                                           in1=tmp2[:], op0=OP.mult, op1=OP.add)
            nc.vector.scalar_tensor_tensor(out=tmp2[:], in0=eAB, scalar=aw4,
                                           in1=tmp1[:], op0=OP.mult, op1=OP.add)
            nc.scalar.activation(tmp1[:], tmp2[:], AF.Sigmoid, bias=cst[:, 0:1])
            nc.vector.tensor_scalar_mul(al2b[:], tmp1[:], float(c_vo))
            nc.scalar.copy(emb[:], em4[:])         # f32 -> bf16 planes
            # xal = xb * (alpha*c_vo) broadcast over v  (one-time, Pool)
            nc.gpsimd.tensor_mul(
                out=tap(xal, 0, [[V, GALL], [1, V]]),
                in0=tap(xb, 0, [[V, GALL], [1, V]]),
                in1=tap(al2b, 0, [[1, GALL], [0, V]]),
            )

            def emit_evac_backs(psths):
                # evac PSUM -> dxT (f16) + XBAR back, both halves; returns
                # dxsm. h0 evac on ACT, h1 on DVE to balance engine load.
                dxsm = iop.tile([P, CPK], F16, tag="dxsm")
                for h in range(2):
                    if h == 0:
                        nc.scalar.copy(
                            dxT[0:120, h * CPKH:(h + 1) * CPKH], psths[h][:])
                    else:
                        nc.vector.tensor_copy(
                            dxT[0:120, h * CPKH:(h + 1) * CPKH], psths[h][:])
                    nc.sync.dma_start_transpose(
                        out=tap(dxsm, h * CPKH, [[P, NBH], [1, P]]).rearrange(
                            "p di m -> p di m"),
                        in_=dxT[:, h * CPKH:(h + 1) * CPKH],
                    )
                return dxsm

            def emit_final(pt3, pdx):
                # final add (in-place into pt3); store DMA emitted later
                nc.gpsimd.tensor_add(
                    out=tap(pt3, 0, [[SLOT * V, NBF], [V, SLOT], [1, V]]),
                    in0=tap(pt3, 0, [[SLOT * V, NBF], [V, SLOT], [1, V]]),
                    in1=tap(pdx, 0, [[P, NBF], [V, SLOT], [1, V]]),
                )
                if TAIL:
                    nc.gpsimd.tensor_add(
                        out=tap(pt3, NBF * SLOT * V, [[V, TAIL], [1, V]]),
                        in0=tap(pt3, NBF * SLOT * V, [[V, TAIL], [1, V]]),
                        in1=tap(pdx, NBF * P, [[V, TAIL], [1, V]]),
                    )
                return pt3


                # ======== dxdt pipeline, two block-halves ========
                psths = []
                for h in range(2):
                    b0 = h * NBH
                    nfull = NBH if h == 0 else NBF - NBH  # 11 / 10 full blocks
                    zxh, zxTh = zxhs[h], zxThs[h]
                    # k0: shuffle x into slot-major (ACT copy); rep 0's
                    # shuffles were pre-emitted ahead of the trig chain
                    if rep > 0:
                        emit_shuffle(h)
                    # k=1..4: zx_k = zx_k0 * em_(k-1), 2x f16 DVE
                    for k in range(1, NSEC):
                        nc.vector.tensor_mul(
                            out=tap(zxh, k * CPKH,
                                    [[P, nfull], [SLOT, V], [1, SLOT]]),
                            in0=tap(zxh, 0, [[P, nfull], [SLOT, V], [1, SLOT]]),
                            in1=tap(emb, (k - 1) * GALL + b0 * SLOT,
                                    [[SLOT, nfull], [0, V], [1, SLOT]]),
                        )
                        if h == 1 and TAIL:
                            nc.vector.tensor_mul(
                                out=tap(zxh, k * CPKH + nfull * P,
                                        [[SLOT, V], [1, TAIL]]),
                                in0=tap(zxh, nfull * P, [[SLOT, V], [1, TAIL]]),
                                in1=tap(emb, (k - 1) * GALL + NBF * SLOT,
                                        [[0, V], [1, TAIL]]),
                            )
                    # XBAR transpose half
                    nc.sync.dma_start_transpose(
                        out=zxTh[:].rearrange("p (di m) -> p di m",
                                              di=NCOLH // P),
                        in_=zxh[:],
                    )
                    # 5 accumulating matmuls per PSUM bank chunk
                    psth = psp.tile([120, CPKH], F32, tag=f"ps{h}")
                    for k in range(NSEC):
                        for c0 in range(0, CPKH, 512):
                            c1 = min(c0 + 512, CPKH)
                            nc.tensor.matmul(
                                psth[:, c0:c1],
                                w5[:, k * 120:(k + 1) * 120],
                                zxTh[:, k * CPKH + c0:k * CPKH + c1],
                                start=(k == 0), stop=(k == NSEC - 1),
                            )
                    psths.append(psth)

                # ======== attention pass 1 + both exps ========
                ablks = emit_ablk()
                eblks = emit_eblk(ablks)
                dnall = ap_.tile([P, GALL * V], BF16, tag="dn")
                for ch in range(NCH):
                    ao = ch * GA * V
                    eblk = eblks[ch]
                    # den = e0 + e1, then += ed on v>=2 (in-place)
                    edeng = nc.gpsimd if "den" in pool_cfg else nc.vector
                    edeng.tensor_add(
                        out=tap(dnall, ao, [[V, GA], [1, V]]),
                        in0=tap(eblk, 0, [[28, GA], [1, V]]),
                        in1=tap(eblk, V, [[28, GA], [1, V]]),
                    )
                    edeng.tensor_add(
                        out=tap(dnall, ao + 2, [[V, GA], [1, 8]]),
                        in0=tap(dnall, ao + 2, [[V, GA], [1, 8]]),
                        in1=tap(eblk, 2 * V, [[28, GA], [1, 8]]),
                    )
                nss = []
                for ch in range(NCH):
                    ao = ch * GA * V
                    eblk = eblks[ch]
                    # s01 = e01 * xal_u (broadcast -> Pool, f32 out)
                    sblk = ap1.tile([P, GA * 2 * V], F32, tag="sblk")
                    (nc.gpsimd if "s01" in pool_cfg else nc.vector).tensor_mul(
                        out=tap(sblk, 0, [[2 * V, GA], [V, 2], [1, V]]),
                        in0=tap(eblk, 0, [[28, GA], [V, 2], [1, V]]),
                        in1=tap(xal, ao, [[V, GA], [1, 2], [0, V]]),
                    )
                    # ss = s0 + s1; += ed*xal_v on v>=2
                    ns = ap_.tile([P, GA * V], F32, tag="ns")
                    (nc.gpsimd if "ss" in pool_cfg else nc.vector).tensor_add(
                        out=tap(ns, 0, [[V, GA], [1, V]]),
                        in0=tap(sblk, 0, [[2 * V, GA], [1, V]]),
                        in1=tap(sblk, V, [[2 * V, GA], [1, V]]),
                    )
                    pdt = ap_.tile([P, GA * 8], F32, tag="pdt")
                    (nc.gpsimd if "pd" in pool_cfg else nc.vector).tensor_mul(
                        out=tap(pdt, 0, [[8, GA], [1, 8]]),
                        in0=tap(eblk, 2 * V, [[28, GA], [1, 8]]),
                        in1=tap(xal, ao + 2, [[V, GA], [1, 8]]),
                    )
                    (nc.gpsimd if "nsp" in pool_cfg else nc.vector).tensor_add(
                        out=tap(ns, 2, [[V, GA], [1, 8]]),
                        in0=tap(ns, 2, [[V, GA], [1, 8]]),
                        in1=tap(pdt, 0, [[8, GA], [1, 8]]),
                    )
                    nss.append(ns)

                # ---- deferred final add for rep-1 (store DMAs emitted at
                # body end so they cannot head-of-line block ld/rd) ----
                pend_dma = emit_final(*prev) if prev is not None else None

                # 1/den via exp(-ln(den)), same ACT table; both chunks at once
                ld = ap1.tile([P, GALL * V], F32, tag="ld")
                nc.scalar.activation(ld[:], dnall[:], AF.Ln)
                rd = ap1.tile([P, GALL * V], F32, tag="rd")
                nc.scalar.activation(rd[:], ld[:], AF.Exp, scale=-1.0)
                t3 = iop.tile([P, GALL * V], F16, tag="t3")
                for ch, ns in enumerate(nss):
                    ao = ch * GA * V
                    # t3 = ns * rd
                    (nc.gpsimd if "q2" in pool_cfg else nc.vector).tensor_mul(
                        out=tap(t3, ao, [[V, GA], [1, V]]),
                        in0=tap(ns, 0, [[V, GA], [1, V]]),
                        in1=tap(rd, ao, [[V, GA], [1, V]]),
                    )
                if pend_dma is not None:
                    nc.sync.dma_start(
                        out=dram_ap(out_d, 0,
                                    [[GALL * V, P], [1, GALL * V]]),
                        in_=pend_dma[:],
                    )
                pend_ps, pend_t3 = psths, t3
            if pend_ps is not None:
                last_t3 = emit_final(pend_t3, emit_evac_backs(pend_ps))
                nc.sync.dma_start(
                    out=dram_ap(out_d, 0, [[GALL * V, P], [1, GALL * V]]),
                    in_=last_t3[:],
                )
    _hoist_excess_waits(nc)
    return nc


def _host_prep(L_basis, wq, wk, wv, wo, alpha_w):
    D = wq.shape[0]
    c_qk = float(np.dot(np.asarray(wq, np.float64), np.asarray(wk, np.float64))
                 / math.sqrt(D))
    c_vo = float(np.dot(np.asarray(wv, np.float64), np.asarray(wo, np.float64)))

    L = np.asarray(L_basis, np.float64)
    # emb features: eA = sin^2(pi t), eB = sin*cos, eA2 = eA^2, eAB = eA*eB
    # cos(2pi t) = 1 - 2 eA ; sin(2pi t) = 2 eB
    # cos(4pi t) = 1 - 8 eA + 8 eA2 ; sin(4pi t) = 4 eB - 8 eAB
    L0 = L[0] + L[1] + L[3]
    L1 = -2.0 * L[1] - 8.0 * L[3]
    L2 = 2.0 * L[2] + 4.0 * L[4]
    L3 = 8.0 * L[3]
    L4 = -8.0 * L[4]
    aw = np.asarray(alpha_w, np.float64)
    a0 = float(aw[0] + aw[1] + aw[3])
    aw1 = float(-2.0 * aw[1] - 8.0 * aw[3])
    aw2 = float(2.0 * aw[2] + 4.0 * aw[4])
    aw3 = float(8.0 * aw[3])
    aw4 = float(-8.0 * aw[4])

    # W5 [128, 5*120]: row p = v*SLOT + slot, col q = k*120 + slot*V + u
    w5 = np.zeros((P, NSEC * 120), np.float64)
    for k, Lk in enumerate([L0, L1, L2, L3, L4]):
        for slot in range(SLOT):
            for v in range(V):
                for u in range(V):
                    w5[v * SLOT + slot, k * 120 + slot * V + u] = Lk[u, v]
    cst = np.array([a0, math.pi / 2.0, 0.0, 0.0], np.float32)
    return dict(c_qk=c_qk, c_vo=c_vo, aw=(aw1, aw2, aw3, aw4),
                w5=w5.astype(np.float16), cst=cst)


def make_in_maps(x, t_years, hp):
    xb = np.asarray(x, np.float32).astype(np.float16)
    t = np.asarray(t_years, np.float32)
    in_maps = []
    for i in range(NCORES):
        sl = slice(i * BC, (i + 1) * BC)
        xc = xb[sl]
        xe = np.ascontiguousarray(
            np.broadcast_to(xc[:, :2, None], (BC, 2, V)).reshape(BC, 2 * V))
        in_maps.append({
            "xb": np.ascontiguousarray(xc),
            "xe": xe,
            "t": np.ascontiguousarray(t[sl]),
            "w5": hp["w5"], "cst": hp["cst"],
        })
    return in_maps


def kernel(x, t_years, L_basis, wq, wk, wv, wo, alpha_w):
    from concourse.bass_utils import run_bass_kernel_spmd

    hp = _host_prep(L_basis, wq, wk, wv, wo, alpha_w)
    nc = build_program(BC, hp["c_qk"], hp["c_vo"], *hp["aw"])
    in_maps = make_in_maps(x, t_years, hp)
    r = run_bass_kernel_spmd(nc, in_maps, core_ids=list(range(NCORES)))
    return np.concatenate(
        [np.asarray(r.results[i]["out"], np.float32) for i in range(NCORES)],
        axis=0)


# revision 9
# speedup vs baseline: 4.5498x; 4.5498x over previous
"""Trainium2 Bass kernel for nn_NXROAttentiveModel (v2).

Exact algebraic reduction of the reference (rank-1 QKV => 3-term masked
softmax; Fourier emb folded into 5 basis matrices host-side). Pure data
parallel over 8 cores; per core bc=32768 = 128 partitions x 256 samples,
sample s = p*256 + col.

Per-rep structure (engines balanced ~16-17us each in CoreSim;
~16.9us/rep steady-state, 91% SP occupancy):
  - dxdt via PE, two block-halves: zx [128, 5*11*128] f16 slot-major
    blocks (12 samples per 128-col block, col = v*12+slot) so the
    4 feature muls run in DVE 2x_1p mode; k0 slot-shuffle is an ACT
    copy. One XBAR transpose per half -> zxT; 5 stationaries W5
    [128,120] accumulate into PSUM [120,1408]; one ACT evac -> dxT f16;
    XBAR back -> dxsm (block col = slot*10+u). Evac/back-transpose and
    the final add + store are software-pipelined one rep behind.
  - attention: 2 chunks of 128 samples; exp args f16 (softmax ratio is
    robust to arg rounding), exp outputs bf16 (range), numerator chain
    f32 on Pool (broadcast APs cannot use DVE 2x anyway), denominator
    bf16 on DVE 2x; 1/den via exp(-ln(den)) keeps Exp/Ln/Square/Copy in
    one ACT table (natural_log_exp) => no table reloads in steady state.
  - alpha folded into xal = x*sigmoid(emb@aw)*c_vo once per execution;
    x01e = x_{0,1} broadcast-expanded host-side (input xe) so a01 runs
    2x on DVE; input DMAs split across the SP and ACT HWDGE queues.
  - fp16 in/out tensors halve HBM + host-link traffic; the final add
    (Pool) writes f16 t3 in-place and a plain SP DMA stores it — the
    store must live on SP: on ACT/Pool it head-of-line blocks the next
    rep's shuffles/attention ops behind the final-add dependency. PSUM
    evac is split ACT/DVE; evac, back-transpose, final add and store
    for rep r are emitted one rep behind.
Max |err| vs f64 reference ~2.7e-3 of absmax on HW.
"""
import sys

sys.path.insert(0, "/opt/trn_rl_repo")

import math
import numpy as np
import concourse.bass as bass
import concourse.mybir as mybir
from concourse import tile

V = 10
P = 128
AF = mybir.ActivationFunctionType
OP = mybir.AluOpType
F32 = mybir.dt.float32
BF16 = mybir.dt.bfloat16
F16 = mybir.dt.float16
PI = math.pi

B_FULL = 262144
NCORES = 8
BC = B_FULL // NCORES          # 32768 per core
GALL = BC // P                 # 256 samples per partition
SLOT = 12                      # samples per 128-col block
NBF = GALL // SLOT             # 21 full blocks
TAIL = GALL - NBF * SLOT       # 4 tail samples (block 21, slots 0..3)
NB = NBF + 1                   # 22 blocks
CPK = NB * P                   # cols per k-section = 2816
NSEC = 5
NCOL = NSEC * CPK              # zx cols = 14080
GA = 128                       # samples per attention chunk
NCH = GALL // GA               # 2 chunks


def _hoist_excess_waits(nc, max_waits=1):
    """Walrus codegen allows only one fused sem-wait per compute instruction."""
    n = 0
    for blk in nc.main_func.blocks:
        il = blk.instructions
        i = 0
        while i < len(il):
            ins = il[i]
            si = ins.sync_info
            if (si is not None and si.on_wait and len(si.on_wait) > max_waits
                    and not isinstance(ins, mybir.InstEventSemaphore)):
                keep = list(si.on_wait[-max_waits:])
                hoist = list(si.on_wait[:-max_waits])
                for w in hoist:
                    nop = mybir.InstEventSemaphore(name=f"hoistw-{n}", ins=[], outs=[])
                    n += 1
                    nop.engine = ins.engine
                    nop.sync_info = mybir.SyncInfo(on_wait=[w], on_update=[])
                    nc.register_instruction(nop, overwrite=True)
                    il.insert(i, nop)
                    i += 1
                ins.sync_info = mybir.SyncInfo(
                    on_wait=keep, on_update=list(si.on_update))
            i += 1
    return n


def build_program(bc, c_qk, c_vo, aw1, aw2, aw3, aw4, reps=1, pool_cfg=None):
    if pool_cfg is None:
        pool_cfg = {"s01", "ss", "pd", "nsp", "q2"}
    assert bc == BC
    nc = bass.Bass()
    xb_d = nc.dram_tensor("xb", [bc, V], F16, kind="ExternalInput")
    xe_d = nc.dram_tensor("xe", [bc, 2 * V], F16, kind="ExternalInput")
    t_d = nc.dram_tensor("t", [bc], F32, kind="ExternalInput")
    w5_d = nc.dram_tensor("w5", [P, NSEC * 120], F16, kind="ExternalInput")
    cst_d = nc.dram_tensor("cst", [4], F32, kind="ExternalInput")
    out_d = nc.dram_tensor("out", [bc, V], F16, kind="ExternalOutput")

    def dram_ap(d, off, dims):
        return bass.AP(d.tensor if hasattr(d, "tensor") else d, off, dims)

    def tap(tl, off, dims):
        return bass.AP(tl.tensor, off, [tl.ap[0]] + dims)

    with tile.TileContext(nc) as tc:
        with (
            tc.tile_pool(name="res", bufs=1) as rp,
            tc.tile_pool(name="att", bufs=2) as ap_,
            tc.tile_pool(name="att1", bufs=1) as ap1,
            tc.tile_pool(name="io", bufs=2) as iop,
            tc.tile_pool(name="ps", bufs=1, space="PSUM") as psp,
        ):
            # ---- resident tiles ----
            xb = rp.tile([P, GALL * V], F16)      # sample-major x (v-major)
            xal = rp.tile([P, GALL * V], F16)     # x * alpha * c_vo
            x01e = rp.tile([P, GALL * 2 * V], F16)  # x_u bcast over v
            dxT = rp.tile([P, CPK], F16)          # matmul out, feat-major
            NBH = 11                      # blocks per half
            CPKH = NBH * P                # 1408 cols per k-section per half
            NCOLH = NSEC * CPKH           # 7040
            zxhs = [rp.tile([P, NCOLH], F16, name=f"zxh{h}") for h in range(2)]
            zxThs = [rp.tile([P, NCOLH], F16, name=f"zxTh{h}") for h in range(2)]
            emb = rp.tile([P, 4 * GALL], F16)     # c_k planes, bf16
            al2b = rp.tile([P, GALL], F16)
            em4 = rp.tile([P, 4 * GALL], F32)      # f32 planes (trig out)
            sh = rp.tile([P, GALL], F32)
            tmp1 = rp.tile([P, GALL], F32)
            tmp2 = rp.tile([P, GALL], F32)
            w5 = rp.tile([P, NSEC * 120], F16)
            cst = rp.tile([P, 4], F32)

            # ---- one-time: input DMAs + pad memsets ----
            nc.sync.dma_start(
                out=xb[:], in_=dram_ap(xb_d, 0, [[GALL * V, P], [1, GALL * V]]))
            nc.sync.dma_start(out=w5[:], in_=dram_ap(
                w5_d, 0, [[NSEC * 120, P], [1, NSEC * 120]]))
            nc.scalar.dma_start(
                out=x01e[:],
                in_=dram_ap(xe_d, 0, [[GALL * 2 * V, P], [1, GALL * 2 * V]]))
            nc.scalar.dma_start(
                out=tmp1[:], in_=dram_ap(t_d, 0, [[GALL, P], [1, GALL]]))
            nc.scalar.dma_start(out=cst[:], in_=dram_ap(cst_d, 0, [[0, P], [1, 4]]))
            # dxT rows 120..127 are never written by evac; zero once so the
            # back-transpose reads defined data
            nc.gpsimd.memset(dxT[96:128, :], 0.0)
            # zx pad cols (120..127 of each block, all sections) + tail slots
            for h in range(2):
                nc.gpsimd.memset(
                    tap(zxhs[h], 120, [[CPKH, NSEC], [P, NBH], [1, 8]]), 0.0)
            nc.gpsimd.memset(
                tap(zxhs[1], (NBH - 1) * P + TAIL,
                    [[CPKH, NSEC], [SLOT, V], [1, SLOT - TAIL]]), 0.0)

            pend_ps = None   # psth pair from previous rep, evac deferred
            pend_t3 = None   # t3 from previous rep
            for rep in range(reps):
                # ---- deferred evac + back-transposes for rep-1 ----
                prev = None
                if pend_ps is not None:
                    prev = (pend_t3, emit_evac_backs(pend_ps))
                    pend_ps = pend_t3 = None

                if rep == 0:
                    # rep-0 shuffles first so the trig chain cannot
                    # head-of-line block them on ACT
                    emit_shuffle(0)
                    emit_shuffle(1)
                def emit_shuffle(h):
                nfull = NBH if h == 0 else NBF - NBH
                nc.scalar.copy(
                    tap(zxhs[h], 0, [[P, nfull], [SLOT, V], [1, SLOT]]),
                    tap(xb, h * NBH * SLOT * V,
                        [[SLOT * V, nfull], [1, V], [V, SLOT]]),
                )
                if h == 1 and TAIL:
                    nc.scalar.copy(
                        tap(zxhs[h], nfull * P, [[SLOT, V], [1, TAIL]]),
                        tap(xb, NBF * SLOT * V, [[1, V], [V, TAIL]]),
                    )

            def emit_ablk():
                # exp-arg builds, both chunks back-to-back (DVE 2x)
                ablks = []
                for ch in range(NCH):
                    ao = ch * GA * V
                    ablk = ap_.tile([P, GA * 28], F16, tag="ablk")
                    # a01[g,u,v] = x_v * x_u  (2x via pre-expanded x01e)
                    nc.vector.tensor_mul(
                        out=tap(ablk, 0, [[28, GA], [V, 2], [1, V]]),
                        in0=tap(xb, ao, [[V, GA], [0, 2], [1, V]]),
                        in1=tap(x01e, 2 * ao, [[2 * V, GA], [V, 2], [1, V]]),
                    )
                    # xsq[g, v>=2] = x_v^2   (2x)
                    nc.vector.tensor_mul(
                        out=tap(ablk, 2 * V, [[28, GA], [1, 8]]),
                        in0=tap(xb, ao + 2, [[V, GA], [1, 8]]),
                        in1=tap(xb, ao + 2, [[V, GA], [1, 8]]),
                    )
                    ablks.append(ablk)
                return ablks

            def emit_eblk(ablks):
                eblks = []
                for ch in range(NCH):
                    eblk = ap_.tile([P, GA * 28], BF16, tag="eblk")
                    nc.scalar.activation(eblk[:], ablks[ch][:], AF.Exp,
                                         scale=c_qk)
                    eblks.append(eblk)
                return eblks

        # ---- one-time: trig + emb planes + alpha + xal ----
            # tmp1 holds t; sh = sin(pi t); tmp2 = cos(pi t)
            nc.scalar.activation(sh[:], tmp1[:], AF.Sin, scale=PI)
            nc.scalar.activation(tmp2[:], tmp1[:], AF.Sin, scale=-PI,
                                 bias=cst[:, 1:2])
            eA = tap(em4, 0 * GALL, [[1, GALL]])
            eB = tap(em4, 1 * GALL, [[1, GALL]])
            eA2 = tap(em4, 2 * GALL, [[1, GALL]])
            eAB = tap(em4, 3 * GALL, [[1, GALL]])
            nc.scalar.activation(eA, sh[:], AF.Square)
            nc.vector.tensor_mul(out=eB, in0=sh[:], in1=tmp2[:])
            nc.scalar.activation(eA2, eA, AF.Square)
            nc.vector.tensor_mul(out=eAB, in0=eA, in1=eB)
            nc.vector.scalar_tensor_tensor(out=tmp1[:], in0=eA, scalar=aw1,
                                           in1=eA, op0=OP.mult, op1=OP.bypass)
            nc.vector.scalar_tensor_tensor(out=tmp2[:], in0=eB, scalar=aw2,
                                           in1=tmp1[:], op0=OP.mult, op1=OP.add)
            nc.vector.scalar_tensor_tensor(out=tmp1[:], in0=eA2, scalar=aw3,
                                           in1=tmp2[:], op0=OP.mult, op1=OP.add)
            nc.vector.scalar_tensor_tensor(out=tmp2[:], in0=eAB, scalar=aw4,
                                           in1=tmp1[:], op0=OP.mult, op1=OP.add)
            nc.scalar.activation(tmp1[:], tmp2[:], AF.Sigmoid, bias=cst[:, 0:1])
            nc.vector.tensor_scalar_mul(al2b[:], tmp1[:], float(c_vo))
            nc.scalar.copy(emb[:], em4[:])         # f32 -> bf16 planes
            # xal = xb * (alpha*c_vo) broadcast over v  (one-time, Pool)
            nc.gpsimd.tensor_mul(
                out=tap(xal, 0, [[V, GALL], [1, V]]),
                in0=tap(xb, 0, [[V, GALL], [1, V]]),
                in1=tap(al2b, 0, [[1, GALL], [0, V]]),
            )

            def emit_evac_backs(psths):
                # evac PSUM -> dxT (f16) + XBAR back, both halves; returns
                # dxsm. h0 evac on ACT, h1 on DVE to balance engine load.
                dxsm = iop.tile([P, CPK], F16, tag="dxsm")
                for h in range(2):
                    if h == 0:
                        nc.scalar.copy(
                            dxT[0:120, h * CPKH:(h + 1) * CPKH], psths[h][:])
                    else:
                        nc.vector.tensor_copy(
                            dxT[0:120, h * CPKH:(h + 1) * CPKH], psths[h][:])
                    nc.sync.dma_start_transpose(
                        out=tap(dxsm, h * CPKH, [[P, NBH], [1, P]]).rearrange(
                            "p di m -> p di m"),
                        in_=dxT[:, h * CPKH:(h + 1) * CPKH],
                    )
                return dxsm

            def emit_final(pt3, pdx):
                # final add (in-place into pt3); store DMA emitted later
                nc.vector.tensor_add(
                    out=tap(pt3, 0, [[SLOT * V, NBF], [V, SLOT], [1, V]]),
                    in0=tap(pt3, 0, [[SLOT * V, NBF], [V, SLOT], [1, V]]),
                    in1=tap(pdx, 0, [[P, NBF], [V, SLOT], [1, V]]),
                )
                if TAIL:
                    nc.vector.tensor_add(
                        out=tap(pt3, NBF * SLOT * V, [[V, TAIL], [1, V]]),
                        in0=tap(pt3, NBF * SLOT * V, [[V, TAIL], [1, V]]),
                        in1=tap(pdx, NBF * P, [[V, TAIL], [1, V]]),
                    )
                return pt3


                # ======== dxdt pipeline, two block-halves ========
                psths = []
                for h in range(2):
                    b0 = h * NBH
                    nfull = NBH if h == 0 else NBF - NBH  # 11 / 10 full blocks
                    zxh, zxTh = zxhs[h], zxThs[h]
                    # k0: shuffle x into slot-major (ACT copy); rep 0's
                    # shuffles were pre-emitted ahead of the trig chain
                    if rep > 0:
                        emit_shuffle(h)
                    # k=1..4: zx_k = zx_k0 * em_(k-1), 2x f16 DVE
                    for k in range(1, NSEC):
                        nc.vector.tensor_mul(
                            out=tap(zxh, k * CPKH,
                                    [[P, nfull], [SLOT, V], [1, SLOT]]),
                            in0=tap(zxh, 0, [[P, nfull], [SLOT, V], [1, SLOT]]),
                            in1=tap(emb, (k - 1) * GALL + b0 * SLOT,
                                    [[SLOT, nfull], [0, V], [1, SLOT]]),
                        )
                        if h == 1 and TAIL:
                            nc.vector.tensor_mul(
                                out=tap(zxh, k * CPKH + nfull * P,
                                        [[SLOT, V], [1, TAIL]]),
                                in0=tap(zxh, nfull * P, [[SLOT, V], [1, TAIL]]),
                                in1=tap(emb, (k - 1) * GALL + NBF * SLOT,
                                        [[0, V], [1, TAIL]]),
                            )
                    # XBAR transpose half
                    nc.sync.dma_start_transpose(
                        out=zxTh[:].rearrange("p (di m) -> p di m",
                                              di=NCOLH // P),
                        in_=zxh[:],
                    )
                    # 5 accumulating matmuls per PSUM bank chunk
                    psth = psp.tile([120, CPKH], F32, tag=f"ps{h}")
                    for k in range(NSEC):
                        for c0 in range(0, CPKH, 512):
                            c1 = min(c0 + 512, CPKH)
                            nc.tensor.matmul(
                                psth[:, c0:c1],
                                w5[:, k * 120:(k + 1) * 120],
                                zxTh[:, k * CPKH + c0:k * CPKH + c1],
                                start=(k == 0), stop=(k == NSEC - 1),
                            )
                    psths.append(psth)

                # ======== attention pass 1 + both exps ========
                ablks = emit_ablk()
                eblks = emit_eblk(ablks)
                dnall = ap_.tile([P, GALL * V], BF16, tag="dn")
                for ch in range(NCH):
                    ao = ch * GA * V
                    eblk = eblks[ch]
                    # den = e0 + e1, then += ed on v>=2 (in-place)
                    edeng = nc.gpsimd if "den" in pool_cfg else nc.vector
                    edeng.tensor_add(
                        out=tap(dnall, ao, [[V, GA], [1, V]]),
                        in0=tap(eblk, 0, [[28, GA], [1, V]]),
                        in1=tap(eblk, V, [[28, GA], [1, V]]),
                    )
                    edeng.tensor_add(
                        out=tap(dnall, ao + 2, [[V, GA], [1, 8]]),
                        in0=tap(dnall, ao + 2, [[V, GA], [1, 8]]),
                        in1=tap(eblk, 2 * V, [[28, GA], [1, 8]]),
                    )
                nss = []
                for ch in range(NCH):
                    ao = ch * GA * V
                    eblk = eblks[ch]
                    # s01 = e01 * xal_u (broadcast -> Pool, f32 out)
                    sblk = ap1.tile([P, GA * 2 * V], F32, tag="sblk")
                    (nc.gpsimd if "s01" in pool_cfg else nc.vector).tensor_mul(
                        out=tap(sblk, 0, [[2 * V, GA], [V, 2], [1, V]]),
                        in0=tap(eblk, 0, [[28, GA], [V, 2], [1, V]]),
                        in1=tap(xal, ao, [[V, GA], [1, 2], [0, V]]),
                    )
                    # ss = s0 + s1; += ed*xal_v on v>=2
                    ns = ap_.tile([P, GA * V], F32, tag="ns")
                    (nc.gpsimd if "ss" in pool_cfg else nc.vector).tensor_add(
                        out=tap(ns, 0, [[V, GA], [1, V]]),
                        in0=tap(sblk, 0, [[2 * V, GA], [1, V]]),
                        in1=tap(sblk, V, [[2 * V, GA], [1, V]]),
                    )
                    pdt = ap_.tile([P, GA * 8], F32, tag="pdt")
                    (nc.gpsimd if "pd" in pool_cfg else nc.vector).tensor_mul(
                        out=tap(pdt, 0, [[8, GA], [1, 8]]),
                        in0=tap(eblk, 2 * V, [[28, GA], [1, 8]]),
                        in1=tap(xal, ao + 2, [[V, GA], [1, 8]]),
                    )
                    (nc.gpsimd if "nsp" in pool_cfg else nc.vector).tensor_add(
                        out=tap(ns, 2, [[V, GA], [1, 8]]),
                        in0=tap(ns, 2, [[V, GA], [1, 8]]),
                        in1=tap(pdt, 0, [[8, GA], [1, 8]]),
                    )
                    nss.append(ns)

                # ---- deferred final add for rep-1 (store DMAs emitted at
                # body end so they cannot head-of-line block ld/rd) ----
                pend_dma = emit_final(*prev) if prev is not None else None

                # 1/den via exp(-ln(den)), same ACT table; both chunks at once
                ld = ap1.tile([P, GALL * V], F32, tag="ld")
                nc.scalar.activation(ld[:], dnall[:], AF.Ln)
                rd = ap1.tile([P, GALL * V], F32, tag="rd")
                nc.scalar.activation(rd[:], ld[:], AF.Exp, scale=-1.0)
                t3 = iop.tile([P, GALL * V], F16, tag="t3")
                for ch, ns in enumerate(nss):
                    ao = ch * GA * V
                    # t3 = ns * rd
                    (nc.gpsimd if "q2" in pool_cfg else nc.vector).tensor_mul(
                        out=tap(t3, ao, [[V, GA], [1, V]]),
                        in0=tap(ns, 0, [[V, GA], [1, V]]),
                        in1=tap(rd, ao, [[V, GA], [1, V]]),
                    )
                if pend_dma is not None:
                    nc.sync.dma_start(
                        out=dram_ap(out_d, 0,
                                    [[GALL * V, P], [1, GALL * V]]),
                        in_=pend_dma[:],
                    )
                pend_ps, pend_t3 = psths, t3
            if pend_ps is not None:
                last_t3 = emit_final(pend_t3, emit_evac_backs(pend_ps))
                nc.sync.dma_start(
                    out=dram_ap(out_d, 0, [[GALL * V, P], [1, GALL * V]]),
                    in_=last_t3[:],
                )
    _hoist_excess_waits(nc)
    return nc


def _host_prep(L_basis, wq, wk, wv, wo, alpha_w):
    D = wq.shape[0]
    c_qk = float(np.dot(np.asarray(wq, np.float64), np.asarray(wk, np.float64))
                 / math.sqrt(D))
    c_vo = float(np.dot(np.asarray(wv, np.float64), np.asarray(wo, np.float64)))

    L = np.asarray(L_basis, np.float64)
    # emb features: eA = sin^2(pi t), eB = sin*cos, eA2 = eA^2, eAB = eA*eB
    # cos(2pi t) = 1 - 2 eA ; sin(2pi t) = 2 eB
    # cos(4pi t) = 1 - 8 eA + 8 eA2 ; sin(4pi t) = 4 eB - 8 eAB
    L0 = L[0] + L[1] + L[3]
    L1 = -2.0 * L[1] - 8.0 * L[3]
    L2 = 2.0 * L[2] + 4.0 * L[4]
    L3 = 8.0 * L[3]
    L4 = -8.0 * L[4]
    aw = np.asarray(alpha_w, np.float64)
    a0 = float(aw[0] + aw[1] + aw[3])
    aw1 = float(-2.0 * aw[1] - 8.0 * aw[3])
    aw2 = float(2.0 * aw[2] + 4.0 * aw[4])
    aw3 = float(8.0 * aw[3])
    aw4 = float(-8.0 * aw[4])

    # W5 [128, 5*120]: row p = v*SLOT + slot, col q = k*120 + slot*V + u
    w5 = np.zeros((P, NSEC * 120), np.float64)
    for k, Lk in enumerate([L0, L1, L2, L3, L4]):
        for slot in range(SLOT):
            for v in range(V):
                for u in range(V):
                    w5[v * SLOT + slot, k * 120 + slot * V + u] = Lk[u, v]
    cst = np.array([a0, math.pi / 2.0, 0.0, 0.0], np.float32)
    return dict(c_qk=c_qk, c_vo=c_vo, aw=(aw1, aw2, aw3, aw4),
                w5=w5.astype(np.float16), cst=cst)


def make_in_maps(x, t_years, hp):
    xb = np.asarray(x, np.float32).astype(np.float16)
    t = np.asarray(t_years, np.float32)
    in_maps = []
    for i in range(NCORES):
        sl = slice(i * BC, (i + 1) * BC)
        xc = xb[sl]
        xe = np.ascontiguousarray(
            np.broadcast_to(xc[:, :2, None], (BC, 2, V)).reshape(BC, 2 * V))
        in_maps.append({
            "xb": np.ascontiguousarray(xc),
            "xe": xe,
            "t": np.ascontiguousarray(t[sl]),
            "w5": hp["w5"], "cst": hp["cst"],
        })
    return in_maps


def kernel(x, t_years, L_basis, wq, wk, wv, wo, alpha_w):
    from concourse.bass_utils import run_bass_kernel_spmd

    hp = _host_prep(L_basis, wq, wk, wv, wo, alpha_w)
    nc = build_program(BC, hp["c_qk"], hp["c_vo"], *hp["aw"])
    in_maps = make_in_maps(x, t_years, hp)
    r = run_bass_kernel_spmd(nc, in_maps, core_ids=list(range(NCORES)))
    return np.concatenate(
        [np.asarray(r.results[i]["out"], np.float32) for i in range(NCORES)],
        axis=0)
